# revision 26
# baseline (speedup 1.0000x reference)
"""Trainium2 Bass kernel for nn_Encoder_block (B=128,S=512,D=24,H=4,HD=6,DFF=48).

Pure data parallel over batch: 16 batches/core x 8 cores. Per core, batches
run in 4 groups of 4 banded onto the 128 partitions in T-layout ([d, token],
batch p of a group at partitions 32p..32p+24).

v4 design:
  - softmax exp split across ScalarE (native Exp) and VectorE (EXP64_ANT =
    (1+s/64)^64 fused); [128,1024] per-(t,head-pair) PSUM tiles.
  - scores PSUM pool is 3 deep (6 banks); LN-stats and FFN matmuls borrow
    short-lived tiles from the same pool; V projection reuses the UO bank;
    so the scores pipeline never waits more than one exp.
  - softmax tail (denominator bcast via stream_shuffle + fused 1-Newton
    reciprocal*UO custom op + Wo matmul) is software-pipelined one batch
    behind attention; LN/FFN tails are pipelined over the two following
    groups in 8 slots.
  - everything matmul is bf16; LN affine folded into the gp multiply when
    g==1,b==0 (host-detected).
"""

import os
import sys

import numpy as np

for _p in ("/opt/trn_rl_repo", "/opt/trn_rl_repo/concourse"):
    if os.path.isdir(_p) and _p not in sys.path:
        sys.path.insert(0, _p)

import concourse.bass as bass
import concourse.bacc as bacc
import concourse.mybir as mybir
import concourse.tile as tile
from concourse.bass_utils import run_bass_kernel_spmd

F32 = mybir.dt.float32
BF16 = mybir.dt.bfloat16
AF = mybir.ActivationFunctionType
ALU = mybir.AluOpType

B, S, D = 128, 512, 24
H, HD, DFF = 4, 6, 48
EPS = 1e-5
NCORES = 8
NB = B // NCORES
SCALE = 1.0 / np.sqrt(HD)
EXPN = 64
C0EXP = float(SCALE / EXPN)

# exp engine per (batch parity, 2t+slot): "A"=ScalarE, "D"=VectorE; 9A/7D
EXP_ASSIGN = [
    ["A", "D", "D", "A", "A", "D", "A", "D"],
    ["A", "D", "D", "A", "A", "D", "D", "A"],
]
BCAST_MASK = [0] * 32
# seed constants for the BITWISE_NOT reciprocal (see dve_ops.py)
RC0, RC1 = -0.23549792, 2.0017324


def _register_custom_dve_ops():
    import concourse.dve_ops as dve_ops
    from concourse.dve_spec import (
        Spec, Src0, Src1, One, C0, C1, AluOp, Bin, sq, relu, lower, _has_src1,
    )
    from concourse.dve_uop import DveOpSpec

    if getattr(dve_ops, "_ant_encoder_ops", None) is not None:
        return dve_ops._ant_encoder_ops

    def _exp64_ref(in0, in1, s0, s1, imm2):
        return ((1.0 + in0.astype(np.float32) * s0) ** 64).astype(np.float32)

    b = One + Src0 * C0
    for _ in range(6):
        b = sq(b)

    _not = Bin(AluOp.BITWISE_NOT, Src1, Src1)
    _y0 = _not * C0
    _y1 = _y0 * (C1 - Src1 * _y0)

    def _recip_mul_ref(in0, in1, s0, s1, imm2):
        nx = (~in1.view(np.int32)).view(np.float32)
        y0 = nx * s0
        y1 = y0 * (s1 - in1 * y0)
        return (in0.astype(np.float32) * y1).astype(np.float32)

    specs = {
        "EXP64_ANT": Spec(body=b, reference=_exp64_ref),
        "RELU_ADD_ANT": Spec(
            body=relu(Src0) + Src1,
            reference=lambda in0, in1, s0, s1, imm2: np.maximum(
                np.nan_to_num(in0.astype(np.float32), nan=0.0), 0
            )
            + in1,
        ),
        "SUBSQ_ANT": Spec(
            body=Src0 - sq(Src1),
            reference=lambda in0, in1, s0, s1, imm2: in0.astype(np.float32)
            - in1.astype(np.float32) * in1.astype(np.float32),
        ),
        "RECIP_MUL_ANT": Spec(body=Src0 * _y1, reference=_recip_mul_ref),
    }
    ops = {}
    for name, spec in specs.items():
        shas = {}
        for ver in ("v3", "v4"):
            tmp = DveOpSpec(
                name=name, opcode=0, uops=lower(spec, ver=ver), rd1_en=_has_src1(spec)
            )
            shas[ver] = tmp.sha(ver)
        op = dve_ops.DveOp(name, spec, subdim=False, uops_sha=shas)
        dve_ops.OPS.append(op)
        ops[name] = op
    dve_ops._SUB_OPCODE_FOR_NAME.clear()
    dve_ops._SUB_OPCODE_FOR_NAME.update(
        {op.name: dve_ops._CUSTOM_DVE_ROW_BASE + i for i, op in enumerate(dve_ops.OPS)}
    )
    assert max(dve_ops._SUB_OPCODE_FOR_NAME.values()) < 0x20
    dve_ops.CUSTOM_DVE_SPECS.update({n: s for n, s in specs.items()})
    dve_ops._ant_encoder_ops = ops
    return ops


def _host_consts(Wq, Wk, Wv, Wo, W1, W2, g1, b1, g2, b2):
    import ml_dtypes

    bf = ml_dtypes.bfloat16
    c = {}
    wqe = np.zeros((128, 128), np.float32)
    wke = np.zeros((128, 128), np.float32)
    for p in range(4):
        for h in range(H):
            for j in range(HD):
                wqe[32 * p : 32 * p + D, 32 * h + j] = Wq[6 * h + j, :]
                wke[32 * p : 32 * p + D, 32 * h + j] = Wk[6 * h + j, :]
    c["wqe"] = wqe.astype(bf)
    c["wke"] = wke.astype(bf)

    wve = np.zeros((128, 32), np.float32)
    for p in range(4):
        for j in range(D):
            wve[32 * p : 32 * p + D, j] = Wv[j, :]
    c["wve"] = wve.astype(bf)

    woe = np.zeros((128, 32), np.float32)
    for h in range(H):
        for j in range(HD):
            woe[32 * h + 1 + j, 0:D] = Wo[:, 6 * h + j]
    c["woe"] = woe.astype(bf)

    cb1 = np.zeros((128, 128), np.float32)
    cb2 = np.zeros((128, 128), np.float32)
    for p in range(4):
        cb1[32 * p : 32 * p + D, 32 * p] = -1.0 / D
        cb2[32 * p : 32 * p + D, 32 * p] = 1.0 / D
    c["cb1"] = cb1.astype(bf)
    c["cb2"] = cb2.astype(bf)

    w1e = np.zeros((128, 4 * 64), np.float32)
    for p in range(4):
        w1e[32 * p : 32 * p + D, 64 * p : 64 * p + DFF] = W1.T
    c["w1e"] = w1e.astype(bf)

    w2e = np.zeros((128, 2 * 32), np.float32)
    w2e[0:DFF, 0:D] = W2.T
    w2e[64 : 64 + DFF, 32 : 32 + D] = W2.T
    c["w2e"] = w2e.astype(bf)

    gb = np.zeros((128, 4), np.float32)
    for p in range(4):
        gb[32 * p : 32 * p + D, 0] = g1
        gb[32 * p : 32 * p + D, 1] = b1
        gb[32 * p : 32 * p + D, 2] = g2
        gb[32 * p : 32 * p + D, 3] = b2
    c["gb"] = gb
    return c


CONST_SHAPES = {
    "wqe": ((128, 128), BF16),
    "wke": ((128, 128), BF16),
    "wve": ((128, 32), BF16),
    "woe": ((128, 32), BF16),
    "cb1": ((128, 128), BF16),
    "cb2": ((128, 128), BF16),
    "w1e": ((128, 4 * 64), BF16),
    "w2e": ((128, 2 * 32), BF16),
    "gb": ((128, 4), F32),
}


def _pin_act_tables():
    import concourse.bacc as _bacc

    if getattr(_bacc, "_act_tables_pinned", False):
        return
    _orig = _bacc.get_activation_tables

    def _patched(arch):
        tables = dict(_orig(arch))
        keep = "natural_log_exp_and_others"
        for name in list(tables):
            if name != keep and (AF.Exp in tables[name] or AF.Ln in tables[name]):
                tables[name] = set()
        return tables

    _bacc.get_activation_tables = _patched
    _bacc._act_tables_pinned = True


def build_nc(nb: int = NB, trivial_affine: bool = True) -> bass.Bass:
    _pin_act_tables()
    OPS = _register_custom_dve_ops()
    EXP64, RELU_ADD = OPS["EXP64_ANT"], OPS["RELU_ADD_ANT"]
    SUBSQ, RECIP_MUL = OPS["SUBSQ_ANT"], OPS["RECIP_MUL_ANT"]
    ngroups = nb // 4
    nc = bacc.Bacc()
    x_in = nc.dram_tensor("x", [nb, S, D], F32, kind="ExternalInput")
    out = nc.dram_tensor("out", [nb, S, D], F32, kind="ExternalOutput")
    cin = {
        k: nc.dram_tensor(k, list(sh), dt, kind="ExternalInput")
        for k, (sh, dt) in CONST_SHAPES.items()
    }

    with tile.TileContext(nc) as tc:
        import contextlib

        ctx = contextlib.ExitStack()
        with ctx:
            constp = ctx.enter_context(tc.tile_pool(name="consts", bufs=1))
            persp = ctx.enter_context(tc.tile_pool(name="pers", bufs=1))
            xtp = ctx.enter_context(tc.tile_pool(name="xt", bufs=2))
            xbp = ctx.enter_context(tc.tile_pool(name="xb", bufs=2))
            qkbp = ctx.enter_context(tc.tile_pool(name="qkb", bufs=2))
            ep = ctx.enter_context(tc.tile_pool(name="e", bufs=2))
            dbp = ctx.enter_context(tc.tile_pool(name="db", bufs=2))
            otp = ctx.enter_context(tc.tile_pool(name="ot", bufs=2))
            y1p = ctx.enter_context(tc.tile_pool(name="y1", bufs=2))
            ybp = ctx.enter_context(tc.tile_pool(name="yb", bufs=4))
            smp = ctx.enter_context(tc.tile_pool(name="sm", bufs=4))
            bcp = ctx.enter_context(tc.tile_pool(name="bc", bufs=4))
            x1p = ctx.enter_context(tc.tile_pool(name="x1", bufs=2))
            hsp = ctx.enter_context(tc.tile_pool(name="hs", bufs=4))
            fsp = ctx.enter_context(tc.tile_pool(name="fs", bufs=2))
            y2p = ctx.enter_context(tc.tile_pool(name="y2", bufs=2))
            # PSUM: scores/qk/stats/ffn 3x[128,1024]=6 banks, UO 1, WOPS 1
            scp = ctx.enter_context(tc.tile_pool(name="sc", bufs=3, space="PSUM"))
            uop = ctx.enter_context(tc.tile_pool(name="uo", bufs=1, space="PSUM"))
            wop = ctx.enter_context(tc.tile_pool(name="wo", bufs=1, space="PSUM"))

            # group-0 input DMAs go FIRST (before the const loads) so the
            # x data is in flight while the PE warm-up runs.
            U2 = []
            for i in range(2):
                U2.append(persp.tile([128, 16, 32], F32, name=f"u{i}"))
            wsrc = constp.tile([128, S], BF16, name="warm_src")
            nc.vector.memset(wsrc[:, :], 0.0)
            for pp in range(4):
                nc.sync.dma_start(
                    out=U2[0][32 * pp : 32 * pp + 32, :, 0:D],
                    in_=x_in[pp].rearrange("(f c) d -> c f d", c=32),
                )
            C = {}
            for k, (sh, dt) in CONST_SHAPES.items():
                t = constp.tile(list(sh), dt, name=f"c_{k}")
                nc.sync.dma_start(out=t, in_=cin[k][:, :])
                C[k] = t
            eps_t = constp.tile([128, 1], F32, name="c_eps")
            nc.vector.memset(eps_t, EPS)

            # HAM warm-up: dense back-to-back matmuls flip the PE clock gate
            # to 8/8 and bridge until the first scores matmuls are ready (the
            # kernel's own bursts are too short to ever warm it from cold).
            wps = scp.tile([128, 2 * S], F32, name="warm_ps", tag="sc")
            for _ in range(26):
                nc.tensor.matmul(
                    wps[:, 0:S], wsrc[:, 0:128], wsrc[:, :],
                    start=True, stop=True, tile_position=(0, 0),
                )

            def warm(n):
                """Dependency-free dummy LDWEIGHTS fill PE idle gaps so the
                HAM activity monitor never re-throttles the clock. Every real
                matmul reloads its own weights, so these are harmless."""
                for _ in range(n):
                    nc.tensor.ldweights(wsrc[:, 0:128])

            VSB2 = []
            for i in range(2):
                nc.vector.memset(U2[i][:, :, D:32], 0.0)
                v = persp.tile([128, 4, 128], BF16, name=f"vsb{i}")
                nc.gpsimd.memset(v[:, :, :], 1.0)
                VSB2.append(v)

            def bcast_dma(dst, src):
                for h in range(4):
                    row = src[32 * h : 32 * h + 1, :]
                    src_b = bass.AP(
                        tensor=row.tensor,
                        offset=row.offset,
                        ap=[list(row.ap[0]), [0, 32]]
                        + [list(x) for x in row.ap[1:]],
                    )
                    nc.sync.dma_start(
                        out=dst[32 * h : 32 * h + 32, :].rearrange(
                            "p (x q) -> p x q", x=1
                        ),
                        in_=src_b,
                    )

            st = {}

            # ---------------- tail pipeline (8 slots / 2 groups) -------------
            def ln_slotA(Y, tag):
                Yb = ybp.tile([128, S], BF16, name=f"yb_{tag}", tag="yb")
                nc.gpsimd.tensor_copy(Yb[:, :], Y[:, :])
                YSQ = ybp.tile([128, S], BF16, name=f"ysq_{tag}", tag="yb")
                nc.gpsimd.tensor_mul(YSQ[:, :], Y[:, :], Y[:, :])
                return Yb, YSQ

            def ln_slotB(YbQ, tag):
                Yb, YSQ = YbQ
                mps = scp.tile([128, 2 * S], F32, name=f"mps_{tag}", tag="sc")
                nc.tensor.matmul(
                    mps[:, 0:S], C["cb1"][:, :], Yb[:, :],
                    start=True, stop=True, tile_position=(0, 0),
                )
                nc.tensor.matmul(
                    mps[:, S : 2 * S], C["cb2"][:, :], YSQ[:, :],
                    start=True, stop=True, tile_position=(0, 0),
                )
                STT = smp.tile([128, 2 * S], BF16, name=f"stt_{tag}", tag="sm")
                nc.scalar.copy(STT[:, 0:S], mps[:, 0:S])
                VAR = smp.tile([128, S], F32, name=f"var_{tag}", tag="var")
                nc.vector._custom_dve(
                    SUBSQ, out=VAR[:, :], in0=mps[:, S : 2 * S], in1=STT[:, 0:S]
                )
                LNV = smp.tile([128, S], F32, name=f"lnv_{tag}", tag="var")
                nc.scalar.activation(LNV[:, :], VAR[:, :], AF.Ln, bias=eps_t[:, :])
                nc.scalar.activation(STT[:, S : 2 * S], LNV[:, :], AF.Exp, scale=-0.5)
                return STT

            def ln_slotC(Y, STT, OUT, gcol, bcol, tag):
                BC = bcp.tile([128, 2 * S], BF16, name=f"bc_{tag}", tag="bc")
                bcast_dma(BC, STT)
                nc.gpsimd.tensor_add(Y[:, :], Y[:, :], BC[:, 0:S])
                if trivial_affine:
                    nc.gpsimd.tensor_mul(OUT[:, :], Y[:, :], BC[:, S : 2 * S])
                else:
                    nc.gpsimd.tensor_mul(Y[:, :], Y[:, :], BC[:, S : 2 * S])
                    nc.vector.tensor_scalar(
                        OUT[:, :], Y[:, :], gcol, bcol, op0=ALU.mult, op1=ALU.add
                    )

            def slot0(g):
                Y1 = y1p.tile([128, S], F32, name=f"y1_{g}", tag="y1")
                nc.vector.tensor_add(Y1[:, :], st["WOPS"][:, :], st["XT4"][:, :])
                st["Y1"] = Y1
                st["L1A"] = ln_slotA(Y1, f"l1g{g}")

            def slot1(g):
                st["L1B"] = ln_slotB(st["L1A"], f"l1g{g}")

            def slot2(g):
                X1 = x1p.tile([128, S], BF16, name=f"x1_{g}", tag="x1")
                ln_slotC(st["Y1"], st["L1B"], X1,
                         C["gb"][:, 0:1], C["gb"][:, 1:2], f"l1g{g}")
                st["X1"] = X1

            def slot3(g):
                X1 = st["X1"]
                hps = scp.tile([128, 2 * S], F32, name=f"hps_{g}", tag="sc")
                for pair in range(2):
                    for j in range(2):
                        p4 = 2 * pair + j
                        nc.tensor.matmul(
                            hps[64 * j : 64 * j + 64, S * pair : S * (pair + 1)],
                            C["w1e"][:, 64 * p4 : 64 * (p4 + 1)],
                            X1[:, :],
                            start=True, stop=True, tile_position=(0, 64 * j),
                            skip_group_check=True,
                        )
                HS = hsp.tile([128, 2 * S], BF16, name=f"hs_{g}", tag="hs")
                nc.scalar.activation(HS[:, :], hps[:, :], AF.Relu)
                f4t = scp.tile([128, 2 * S], F32, name=f"f4_{g}", tag="sc")
                F4 = f4t[:, 0:S]
                for pair in range(2):
                    for j in range(2):
                        p4 = 2 * pair + j
                        nc.tensor.matmul(
                            F4[32 * p4 : 32 * p4 + 32, :],
                            C["w2e"][:, 32 * j : 32 * (j + 1)],
                            HS[:, S * pair : S * (pair + 1)],
                            start=True, stop=True, tile_position=(0, 32 * p4),
                            skip_group_check=True,
                        )
                FS = fsp.tile([128, S], F32, name=f"fs_{g}", tag="fs")
                nc.vector._custom_dve(
                    RELU_ADD, out=FS[:, :], in0=F4, in1=X1[:, :]
                )
                st["FS"] = FS

            def slot4(g):
                st["L2A"] = ln_slotA(st["FS"], f"l2g{g}")

            def slot5(g):
                st["L2B"] = ln_slotB(st["L2A"], f"l2g{g}")

            def slot6(g):
                Y2N = y2p.tile([128, S], F32, name=f"y2n_{g}", tag="y2n")
                ln_slotC(st["FS"], st["L2B"], Y2N,
                         C["gb"][:, 2:3], C["gb"][:, 3:4], f"l2g{g}")
                st["Y2N"] = Y2N

            def slot7(g):
                Y2T = y2p.tile([128, S], F32, name=f"y2t_{g}", tag="y2t")
                nc.vector.transpose(Y2T[:, :], st["Y2N"][:, :])
                for pp in range(4):
                    nc.sync.dma_start(
                        out=out[4 * g + pp].rearrange("(f r) d -> r f d", r=32),
                        in_=Y2T[32 * pp : 32 * pp + 32, :].rearrange(
                            "r (f c) -> r f c", c=32
                        )[:, :, 0:D],
                    )

            SLOTS = [slot0, slot1, slot2, slot3, slot4, slot5, slot6, slot7]

            def run_tails(gg, p):
                if gg >= 1:
                    SLOTS[p](gg - 1)
                if gg >= 2:
                    SLOTS[4 + p](gg - 2)

            def softmax_tail(bprev):
                """denominator bcast + normalize + Wo for batch bprev."""
                UO = st["UO_prev"]
                DB = dbp.tile([128, S], F32, name=f"db_{bprev}", tag="db")
                nc.vector.stream_shuffle(DB[:, :], UO[:, :], BCAST_MASK)
                OTn = otp.tile([128, S], BF16, name=f"ot_{bprev}", tag="ot")
                nc.vector._custom_dve(
                    RECIP_MUL, out=OTn[:, :], in0=UO[:, :], in1=DB[:, :],
                    s0=RC0, s1=RC1,
                )
                st["OTn_prev"] = OTn

            def wo_mm(bprev):
                gprev, pprev = bprev // 4, bprev % 4
                if pprev == 0:
                    st["WOPS_new"] = wop.tile(
                        [128, S], F32, name=f"wops_{gprev}", tag="wops"
                    )
                nc.tensor.matmul(
                    st["WOPS_new"][32 * pprev : 32 * pprev + 32, :],
                    C["woe"][:, :], st["OTn_prev"][:, :],
                    start=True, stop=True, tile_position=(0, 32 * pprev),
                    skip_group_check=True,
                )
                if pprev == 3:
                    st["WOPS"], st["XT4"] = st["WOPS_new"], st["XT4_keep"]

            # --------------------------- main loop ---------------------------
            for b in range(nb):
                g, p = b // 4, b % 4
                if p == 0:
                    U = U2[g % 2]
                    if g > 0:  # group 0's loads were issued in the preamble
                        for pp in range(4):
                            nc.sync.dma_start(
                                out=U[32 * pp : 32 * pp + 32, :, 0:D],
                                in_=x_in[4 * g + pp].rearrange(
                                    "(f c) d -> c f d", c=32
                                ),
                            )
                    XT4 = xtp.tile([128, S], F32, name=f"xt4_{g}", tag="xt")
                    nc.vector.transpose(XT4[:, :], U.rearrange("P a c -> P (a c)"))
                    XT4b = xbp.tile([128, S], BF16, name=f"xtb_{g}", tag="xb")
                    nc.vector.tensor_copy(XT4b[:, :], XT4[:, :])
                    st["XT4_new"], st["XT4b"] = XT4, XT4b

                XT4b = st["XT4b"]
                # Q/K projections -> one merged bf16 copy
                ps_qk = scp.tile([128, 2 * S], F32, name=f"qk_{b}", tag="sc")
                nc.tensor.matmul(
                    ps_qk[:, 0:S], C["wqe"][32 * p : 32 * p + D, :],
                    XT4b[32 * p : 32 * p + D, :],
                    start=True, stop=True, tile_position=(32 * p, 0),
                )
                nc.tensor.matmul(
                    ps_qk[:, S : 2 * S], C["wke"][32 * p : 32 * p + D, :],
                    XT4b[32 * p : 32 * p + D, :],
                    start=True, stop=True, tile_position=(32 * p, 0),
                )
                # previous batch's AV last chunk + softmax tail fills the gap
                if b > 0:
                    Ep, UOp, VSBp = st["E_prev"], st["UO_prev"], st["VSB_prev"]
                    for h in range(4):
                        nc.tensor.matmul(
                            UOp[32 * h : 32 * h + 32, :],
                            VSBp[:, 3, 32 * h : 32 * h + 32],
                            Ep[:, 3, h, :],
                            start=False, stop=True,
                            tile_position=(0, 32 * h),
                            skip_group_check=True,
                        )
                QKb = qkbp.tile([128, 2 * S], BF16, name=f"qkb_{b}", tag="qkb")
                nc.scalar.copy(QKb[:, :], ps_qk[:, :])
                if b > 0:
                    softmax_tail(b - 1)

                # scores + exp + AV (av(t) emitted after sc(t+1))
                E = ep.tile([128, 4, 4, S], BF16, name=f"e_{b}", tag="e")

                def sc_t(t):
                    for slot in range(2):
                        h0 = 2 * slot
                        SC = scp.tile([128, 2 * S], F32,
                                      name=f"sc{b}_{t}_{slot}", tag="sc")
                        for hh in range(2):
                            h = h0 + hh
                            nc.tensor.matmul(
                                SC[:, S * hh : S * (hh + 1)],
                                QKb[32 * h : 32 * h + HD,
                                    S + 128 * t : S + 128 * (t + 1)],
                                QKb[32 * h : 32 * h + HD, 0:S],
                                start=True, stop=True,
                                tile_position=(32 * h, 0),
                            )
                        edst = E[:, t, h0 : h0 + 2, :]
                        if EXP_ASSIGN[b % 2][2 * t + slot] == "A":
                            nc.scalar.activation(
                                edst, SC[:, :], AF.Exp, scale=float(SCALE)
                            )
                        else:
                            nc.vector._custom_dve(
                                EXP64, out=edst, in0=SC[:, :], s0=C0EXP
                            )

                def av_t(t):
                    for h in range(4):
                        nc.tensor.matmul(
                            UO[32 * h : 32 * h + 32, :],
                            Vsb[:, t, 32 * h : 32 * h + 32],
                            E[:, t, h, :],
                            start=(t == 0), stop=False,
                            tile_position=(0, 32 * h),
                            skip_group_check=True,
                        )

                sc_t(0)
                warm(3)
                # V natural layout into the UO bank (before av(0) clears it)
                UO = uop.tile([128, S], F32, name=f"uo_{b}", tag="uo")
                for cch in range(4):
                    nc.tensor.matmul(
                        UO[:, 32 * cch : 32 * cch + 32],
                        XT4b[32 * p : 32 * p + D, 128 * cch : 128 * (cch + 1)],
                        C["wve"][32 * p : 32 * p + D, :],
                        start=True, stop=True, tile_position=(32 * p, 0),
                    )
                Vsb = VSB2[b % 2]
                nc.vector.tensor_copy(
                    Vsb.rearrange("P t (h m) -> P t h m", m=32)[:, :, :, 1 : 1 + HD],
                    UO.rearrange("P (c x) -> P c x", x=32)[:, 0:4, 0:D].rearrange(
                        "P c (h m) -> P c h m", m=HD
                    ),
                )
                sc_t(1)
                warm(2)
                if b > 0:
                    wo_mm(b - 1)
                av_t(0)
                warm(2)
                sc_t(2)
                warm(2)
                av_t(1)
                warm(2)
                sc_t(3)
                warm(2)
                av_t(2)
                warm(2)
                # av(3) is emitted at the start of the next batch

                run_tails(g, p)
                warm(3)
                st["E_prev"], st["UO_prev"], st["VSB_prev"] = E, UO, Vsb
                if p == 0:
                    st["XT4_keep"] = st["XT4_new"]

            # drain: last batch's av(3) + softmax tail + wo, then tail slots
            b = nb
            Ep, UOp, VSBp = st["E_prev"], st["UO_prev"], st["VSB_prev"]
            for h in range(4):
                nc.tensor.matmul(
                    UOp[32 * h : 32 * h + 32, :],
                    VSBp[:, 3, 32 * h : 32 * h + 32],
                    Ep[:, 3, h, :],
                    start=False, stop=True,
                    tile_position=(0, 32 * h),
                    skip_group_check=True,
                )
            softmax_tail(nb - 1)
            wo_mm(nb - 1)
            for vb in range(nb, nb + 8):
                gg, pp = vb // 4, vb % 4
                if gg >= 1 and gg - 1 < ngroups and vb < nb + 4:
                    SLOTS[pp](gg - 1)
                if gg >= 2 and gg - 2 < ngroups:
                    SLOTS[4 + pp](gg - 2)
    nc.compile()
    return nc


_NC_CACHE: dict = {}


def _get_nc(nb: int, trivial_affine: bool = True) -> bass.Bass:
    key = (nb, trivial_affine)
    if key not in _NC_CACHE:
        _NC_CACHE[key] = build_nc(nb, trivial_affine)
    return _NC_CACHE[key]


def kernel(x, Wq, Wk, Wv, Wo, W1, W2, g1, b1, g2, b2):
    x = np.asarray(x, np.float32)
    args = [np.asarray(a, np.float32) for a in (Wq, Wk, Wv, Wo, W1, W2, g1, b1, g2, b2)]
    consts = _host_consts(*args)
    g1a, b1a, g2a, b2a = args[6], args[7], args[8], args[9]
    trivial = bool(
        np.all(g1a == 1.0) and np.all(b1a == 0.0)
        and np.all(g2a == 1.0) and np.all(b2a == 0.0)
    )
    nc = _get_nc(NB, trivial)
    in_maps = []
    for c in range(NCORES):
        m = {"x": np.ascontiguousarray(x[c * NB : (c + 1) * NB])}
        m.update(consts)
        in_maps.append(m)
    res = run_bass_kernel_spmd(nc, in_maps, list(range(NCORES)))
    return np.concatenate([r["out"] for r in res.results], axis=0)


# revision 27
# speedup vs baseline: 1.0468x; 1.0468x over previous
"""Trainium2 Bass kernel for nn_Encoder_block (B=128,S=512,D=24,H=4,HD=6,DFF=48).

Pure data parallel over batch: 16 batches/core x 8 cores. Per core, batches
run in 4 groups of 4 banded onto the 128 partitions in T-layout ([d, token],
batch p of a group at partitions 32p..32p+24).

v4 design:
  - softmax exp split across ScalarE (native Exp) and VectorE (EXP64_ANT =
    (1+s/64)^64 fused); [128,1024] per-(t,head-pair) PSUM tiles.
  - scores PSUM pool is 3 deep (6 banks); LN-stats and FFN matmuls borrow
    short-lived tiles from the same pool; V projection reuses the UO bank;
    so the scores pipeline never waits more than one exp.
  - softmax tail (denominator bcast via stream_shuffle + fused 1-Newton
    reciprocal*UO custom op + Wo matmul) is software-pipelined one batch
    behind attention; LN/FFN tails are pipelined over the two following
    groups in 8 slots.
  - everything matmul is bf16; LN affine folded into the gp multiply when
    g==1,b==0 (host-detected).
"""

import os
import sys

import numpy as np

for _p in ("/opt/trn_rl_repo", "/opt/trn_rl_repo/concourse"):
    if os.path.isdir(_p) and _p not in sys.path:
        sys.path.insert(0, _p)

import concourse.bass as bass
import concourse.bacc as bacc
import concourse.mybir as mybir
import concourse.tile as tile
from concourse.bass_utils import run_bass_kernel_spmd

F32 = mybir.dt.float32
BF16 = mybir.dt.bfloat16
AF = mybir.ActivationFunctionType
ALU = mybir.AluOpType

B, S, D = 128, 512, 24
H, HD, DFF = 4, 6, 48
EPS = 1e-5
NCORES = 8
NB = B // NCORES
SCALE = 1.0 / np.sqrt(HD)
EXPN = 64
C0EXP = float(SCALE / EXPN)

# exp engine per (batch parity, 2t+slot): "A"=ScalarE, "D"=VectorE; 9A/7D
EXP_ASSIGN = [
    ["A", "D", "D", "A", "A", "D", "A", "A"],
    ["A", "D", "D", "A", "A", "D", "A", "D"],
]
BCAST_MASK = [0] * 32
# seed constants for the BITWISE_NOT reciprocal (see dve_ops.py)
RC0, RC1 = -0.23549792, 2.0017324


def _register_custom_dve_ops():
    import concourse.dve_ops as dve_ops
    from concourse.dve_spec import (
        Spec, Src0, Src1, One, C0, C1, AluOp, Bin, sq, relu, lower, _has_src1,
    )
    from concourse.dve_uop import DveOpSpec

    if getattr(dve_ops, "_ant_encoder_ops", None) is not None:
        return dve_ops._ant_encoder_ops

    def _exp64_ref(in0, in1, s0, s1, imm2):
        return ((1.0 + in0.astype(np.float32) * s0) ** 64).astype(np.float32)

    b = One + Src0 * C0
    for _ in range(6):
        b = sq(b)

    _not = Bin(AluOp.BITWISE_NOT, Src1, Src1)
    _y0 = _not * C0
    _y1 = _y0 * (C1 - Src1 * _y0)

    def _recip_mul_ref(in0, in1, s0, s1, imm2):
        nx = (~in1.view(np.int32)).view(np.float32)
        y0 = nx * s0
        y1 = y0 * (s1 - in1 * y0)
        return (in0.astype(np.float32) * y1).astype(np.float32)

    specs = {
        "EXP64_ANT": Spec(body=b, reference=_exp64_ref),
        "RELU_ADD_ANT": Spec(
            body=relu(Src0) + Src1,
            reference=lambda in0, in1, s0, s1, imm2: np.maximum(
                np.nan_to_num(in0.astype(np.float32), nan=0.0), 0
            )
            + in1,
        ),
        "SUBSQ_ANT": Spec(
            body=Src0 - sq(Src1),
            reference=lambda in0, in1, s0, s1, imm2: in0.astype(np.float32)
            - in1.astype(np.float32) * in1.astype(np.float32),
        ),
        "RECIP_MUL_ANT": Spec(body=Src0 * _y1, reference=_recip_mul_ref),
    }
    ops = {}
    for name, spec in specs.items():
        shas = {}
        for ver in ("v3", "v4"):
            tmp = DveOpSpec(
                name=name, opcode=0, uops=lower(spec, ver=ver), rd1_en=_has_src1(spec)
            )
            shas[ver] = tmp.sha(ver)
        op = dve_ops.DveOp(name, spec, subdim=False, uops_sha=shas)
        dve_ops.OPS.append(op)
        ops[name] = op
    dve_ops._SUB_OPCODE_FOR_NAME.clear()
    dve_ops._SUB_OPCODE_FOR_NAME.update(
        {op.name: dve_ops._CUSTOM_DVE_ROW_BASE + i for i, op in enumerate(dve_ops.OPS)}
    )
    assert max(dve_ops._SUB_OPCODE_FOR_NAME.values()) < 0x20
    dve_ops.CUSTOM_DVE_SPECS.update({n: s for n, s in specs.items()})
    dve_ops._ant_encoder_ops = ops
    return ops


def _host_consts(Wq, Wk, Wv, Wo, W1, W2, g1, b1, g2, b2):
    import ml_dtypes

    bf = ml_dtypes.bfloat16
    c = {}
    wqe = np.zeros((128, 128), np.float32)
    wke = np.zeros((128, 128), np.float32)
    for p in range(4):
        for h in range(H):
            for j in range(HD):
                wqe[32 * p : 32 * p + D, 32 * h + j] = Wq[6 * h + j, :]
                wke[32 * p : 32 * p + D, 32 * h + j] = Wk[6 * h + j, :]
    c["wqe"] = wqe.astype(bf)
    c["wke"] = wke.astype(bf)

    wve = np.zeros((128, 32), np.float32)
    for p in range(4):
        for j in range(D):
            wve[32 * p : 32 * p + D, j] = Wv[j, :]
    c["wve"] = wve.astype(bf)

    woe = np.zeros((128, 32), np.float32)
    for h in range(H):
        for j in range(HD):
            woe[32 * h + 1 + j, 0:D] = Wo[:, 6 * h + j]
    c["woe"] = woe.astype(bf)

    cb1 = np.zeros((128, 128), np.float32)
    cb2 = np.zeros((128, 128), np.float32)
    for p in range(4):
        cb1[32 * p : 32 * p + D, 32 * p] = -1.0 / D
        cb2[32 * p : 32 * p + D, 32 * p] = 1.0 / D
    c["cb1"] = cb1.astype(bf)
    c["cb2"] = cb2.astype(bf)

    w1e = np.zeros((128, 4 * 64), np.float32)
    for p in range(4):
        w1e[32 * p : 32 * p + D, 64 * p : 64 * p + DFF] = W1.T
    c["w1e"] = w1e.astype(bf)

    w2e = np.zeros((128, 2 * 32), np.float32)
    w2e[0:DFF, 0:D] = W2.T
    w2e[64 : 64 + DFF, 32 : 32 + D] = W2.T
    c["w2e"] = w2e.astype(bf)

    gb = np.zeros((128, 4), np.float32)
    for p in range(4):
        gb[32 * p : 32 * p + D, 0] = g1
        gb[32 * p : 32 * p + D, 1] = b1
        gb[32 * p : 32 * p + D, 2] = g2
        gb[32 * p : 32 * p + D, 3] = b2
    c["gb"] = gb
    return c


CONST_SHAPES = {
    "wqe": ((128, 128), BF16),
    "wke": ((128, 128), BF16),
    "wve": ((128, 32), BF16),
    "woe": ((128, 32), BF16),
    "cb1": ((128, 128), BF16),
    "cb2": ((128, 128), BF16),
    "w1e": ((128, 4 * 64), BF16),
    "w2e": ((128, 2 * 32), BF16),
    "gb": ((128, 4), F32),
}


def _pin_act_tables():
    import concourse.bacc as _bacc

    if getattr(_bacc, "_act_tables_pinned", False):
        return
    _orig = _bacc.get_activation_tables

    def _patched(arch):
        tables = dict(_orig(arch))
        keep = "natural_log_exp_and_others"
        for name in list(tables):
            if name != keep and (AF.Exp in tables[name] or AF.Ln in tables[name]):
                tables[name] = set()
        return tables

    _bacc.get_activation_tables = _patched
    _bacc._act_tables_pinned = True


def build_nc(nb: int = NB, trivial_affine: bool = True) -> bass.Bass:
    _pin_act_tables()
    OPS = _register_custom_dve_ops()
    EXP64, RELU_ADD = OPS["EXP64_ANT"], OPS["RELU_ADD_ANT"]
    SUBSQ, RECIP_MUL = OPS["SUBSQ_ANT"], OPS["RECIP_MUL_ANT"]
    ngroups = nb // 4
    nc = bacc.Bacc()
    x_in = nc.dram_tensor("x", [nb, S, D], F32, kind="ExternalInput")
    out = nc.dram_tensor("out", [nb, S, D], F32, kind="ExternalOutput")
    cin = {
        k: nc.dram_tensor(k, list(sh), dt, kind="ExternalInput")
        for k, (sh, dt) in CONST_SHAPES.items()
    }

    with tile.TileContext(nc) as tc:
        import contextlib

        ctx = contextlib.ExitStack()
        with ctx:
            constp = ctx.enter_context(tc.tile_pool(name="consts", bufs=1))
            persp = ctx.enter_context(tc.tile_pool(name="pers", bufs=1))
            xtp = ctx.enter_context(tc.tile_pool(name="xt", bufs=2))
            xbp = ctx.enter_context(tc.tile_pool(name="xb", bufs=2))
            qkbp = ctx.enter_context(tc.tile_pool(name="qkb", bufs=2))
            ep = ctx.enter_context(tc.tile_pool(name="e", bufs=2))
            dbp = ctx.enter_context(tc.tile_pool(name="db", bufs=2))
            otp = ctx.enter_context(tc.tile_pool(name="ot", bufs=2))
            y1p = ctx.enter_context(tc.tile_pool(name="y1", bufs=2))
            ybp = ctx.enter_context(tc.tile_pool(name="yb", bufs=4))
            smp = ctx.enter_context(tc.tile_pool(name="sm", bufs=4))
            bcp = ctx.enter_context(tc.tile_pool(name="bc", bufs=4))
            x1p = ctx.enter_context(tc.tile_pool(name="x1", bufs=2))
            hsp = ctx.enter_context(tc.tile_pool(name="hs", bufs=4))
            fsp = ctx.enter_context(tc.tile_pool(name="fs", bufs=2))
            y2p = ctx.enter_context(tc.tile_pool(name="y2", bufs=2))
            # PSUM: scores/qk/stats/ffn 3x[128,1024]=6 banks, UO 1, WOPS 1
            scp = ctx.enter_context(tc.tile_pool(name="sc", bufs=3, space="PSUM"))
            uop = ctx.enter_context(tc.tile_pool(name="uo", bufs=1, space="PSUM"))
            wop = ctx.enter_context(tc.tile_pool(name="wo", bufs=1, space="PSUM"))

            # group-0 input DMAs go FIRST (before the const loads) so the
            # x data is in flight while the PE warm-up runs.
            U2 = []
            for i in range(2):
                U2.append(persp.tile([128, 16, 32], F32, name=f"u{i}"))
            wsrc = constp.tile([128, S], BF16, name="warm_src")
            nc.vector.memset(wsrc[:, :], 0.0)
            for pp in range(4):
                nc.sync.dma_start(
                    out=U2[0][32 * pp : 32 * pp + 32, :, 0:D],
                    in_=x_in[pp].rearrange("(f c) d -> c f d", c=32),
                )
            C = {}
            for k, (sh, dt) in CONST_SHAPES.items():
                t = constp.tile(list(sh), dt, name=f"c_{k}")
                nc.sync.dma_start(out=t, in_=cin[k][:, :])
                C[k] = t
            eps_t = constp.tile([128, 1], F32, name="c_eps")
            nc.vector.memset(eps_t, EPS)

            # HAM warm-up: dense back-to-back matmuls flip the PE clock gate
            # to 8/8 and bridge until the first scores matmuls are ready (the
            # kernel's own bursts are too short to ever warm it from cold).
            wps = scp.tile([128, 2 * S], F32, name="warm_ps", tag="sc")
            for _ in range(26):
                nc.tensor.matmul(
                    wps[:, 0:S], wsrc[:, 0:128], wsrc[:, :],
                    start=True, stop=True, tile_position=(0, 0),
                )

            def warm(n):
                """Dependency-free dummy LDWEIGHTS fill PE idle gaps so the
                HAM activity monitor never re-throttles the clock. Every real
                matmul reloads its own weights, so these are harmless."""
                for _ in range(n):
                    nc.tensor.ldweights(wsrc[:, 0:128])

            VSB2 = []
            for i in range(2):
                nc.vector.memset(U2[i][:, :, D:32], 0.0)
                v = persp.tile([128, 4, 128], BF16, name=f"vsb{i}")
                nc.gpsimd.memset(v[:, :, :], 1.0)
                VSB2.append(v)

            def bcast_dma(dst, src):
                for h in range(4):
                    row = src[32 * h : 32 * h + 1, :]
                    src_b = bass.AP(
                        tensor=row.tensor,
                        offset=row.offset,
                        ap=[list(row.ap[0]), [0, 32]]
                        + [list(x) for x in row.ap[1:]],
                    )
                    nc.sync.dma_start(
                        out=dst[32 * h : 32 * h + 32, :].rearrange(
                            "p (x q) -> p x q", x=1
                        ),
                        in_=src_b,
                    )

            st = {}

            # ---------------- tail pipeline (8 slots / 2 groups) -------------
            def ln_slotA(Y, tag):
                Yb = ybp.tile([128, S], BF16, name=f"yb_{tag}", tag="yb")
                nc.gpsimd.tensor_copy(Yb[:, :], Y[:, :])
                YSQ = ybp.tile([128, S], BF16, name=f"ysq_{tag}", tag="yb")
                nc.gpsimd.tensor_mul(YSQ[:, :], Y[:, :], Y[:, :])
                return Yb, YSQ

            def ln_slotB(YbQ, tag):
                Yb, YSQ = YbQ
                mps = scp.tile([128, 2 * S], F32, name=f"mps_{tag}", tag="sc")
                nc.tensor.matmul(
                    mps[:, 0:S], C["cb1"][:, :], Yb[:, :],
                    start=True, stop=True, tile_position=(0, 0),
                )
                nc.tensor.matmul(
                    mps[:, S : 2 * S], C["cb2"][:, :], YSQ[:, :],
                    start=True, stop=True, tile_position=(0, 0),
                )
                STT = smp.tile([128, 2 * S], BF16, name=f"stt_{tag}", tag="sm")
                nc.scalar.copy(STT[:, 0:S], mps[:, 0:S])
                VAR = smp.tile([128, S], F32, name=f"var_{tag}", tag="var")
                nc.vector._custom_dve(
                    SUBSQ, out=VAR[:, :], in0=mps[:, S : 2 * S], in1=STT[:, 0:S]
                )
                LNV = smp.tile([128, S], F32, name=f"lnv_{tag}", tag="var")
                nc.scalar.activation(LNV[:, :], VAR[:, :], AF.Ln, bias=eps_t[:, :])
                nc.scalar.activation(STT[:, S : 2 * S], LNV[:, :], AF.Exp, scale=-0.5)
                return STT

            def ln_slotC(Y, STT, OUT, gcol, bcol, tag):
                BC = bcp.tile([128, 2 * S], BF16, name=f"bc_{tag}", tag="bc")
                bcast_dma(BC, STT)
                nc.gpsimd.tensor_add(Y[:, :], Y[:, :], BC[:, 0:S])
                if trivial_affine:
                    nc.gpsimd.tensor_mul(OUT[:, :], Y[:, :], BC[:, S : 2 * S])
                else:
                    nc.gpsimd.tensor_mul(Y[:, :], Y[:, :], BC[:, S : 2 * S])
                    nc.vector.tensor_scalar(
                        OUT[:, :], Y[:, :], gcol, bcol, op0=ALU.mult, op1=ALU.add
                    )

            def slot0(g):
                Y1 = y1p.tile([128, S], F32, name=f"y1_{g}", tag="y1")
                nc.vector.tensor_add(Y1[:, :], st["WOPS"][:, :], st["XT4"][:, :])
                st["Y1"] = Y1
                st["L1A"] = ln_slotA(Y1, f"l1g{g}")

            def slot1(g):
                st["L1B"] = ln_slotB(st["L1A"], f"l1g{g}")

            def slot2(g):
                X1 = x1p.tile([128, S], BF16, name=f"x1_{g}", tag="x1")
                ln_slotC(st["Y1"], st["L1B"], X1,
                         C["gb"][:, 0:1], C["gb"][:, 1:2], f"l1g{g}")
                st["X1"] = X1

            def slot3(g):
                X1 = st["X1"]
                hps = scp.tile([128, 2 * S], F32, name=f"hps_{g}", tag="sc")
                for pair in range(2):
                    for j in range(2):
                        p4 = 2 * pair + j
                        nc.tensor.matmul(
                            hps[64 * j : 64 * j + 64, S * pair : S * (pair + 1)],
                            C["w1e"][:, 64 * p4 : 64 * (p4 + 1)],
                            X1[:, :],
                            start=True, stop=True, tile_position=(0, 64 * j),
                            skip_group_check=True,
                        )
                HS = hsp.tile([128, 2 * S], BF16, name=f"hs_{g}", tag="hs")
                nc.scalar.activation(HS[:, :], hps[:, :], AF.Relu)
                f4t = scp.tile([128, 2 * S], F32, name=f"f4_{g}", tag="sc")
                F4 = f4t[:, 0:S]
                for pair in range(2):
                    for j in range(2):
                        p4 = 2 * pair + j
                        nc.tensor.matmul(
                            F4[32 * p4 : 32 * p4 + 32, :],
                            C["w2e"][:, 32 * j : 32 * (j + 1)],
                            HS[:, S * pair : S * (pair + 1)],
                            start=True, stop=True, tile_position=(0, 32 * p4),
                            skip_group_check=True,
                        )
                FS = fsp.tile([128, S], F32, name=f"fs_{g}", tag="fs")
                nc.vector._custom_dve(
                    RELU_ADD, out=FS[:, :], in0=F4, in1=X1[:, :]
                )
                st["FS"] = FS

            def slot4(g):
                st["L2A"] = ln_slotA(st["FS"], f"l2g{g}")

            def slot5(g):
                st["L2B"] = ln_slotB(st["L2A"], f"l2g{g}")

            def slot6(g):
                Y2N = y2p.tile([128, S], F32, name=f"y2n_{g}", tag="y2n")
                ln_slotC(st["FS"], st["L2B"], Y2N,
                         C["gb"][:, 2:3], C["gb"][:, 3:4], f"l2g{g}")
                st["Y2N"] = Y2N

            def slot7(g):
                Y2T = y2p.tile([128, S], F32, name=f"y2t_{g}", tag="y2t")
                nc.vector.transpose(Y2T[:, :], st["Y2N"][:, :])
                for pp in range(4):
                    nc.sync.dma_start(
                        out=out[4 * g + pp].rearrange("(f r) d -> r f d", r=32),
                        in_=Y2T[32 * pp : 32 * pp + 32, :].rearrange(
                            "r (f c) -> r f c", c=32
                        )[:, :, 0:D],
                    )

            SLOTS = [slot0, slot1, slot2, slot3, slot4, slot5, slot6, slot7]

            def run_tails(gg, p):
                if gg >= 1:
                    SLOTS[p](gg - 1)
                if gg >= 2:
                    SLOTS[4 + p](gg - 2)

            def softmax_tail(bprev):
                """denominator bcast + normalize + Wo for batch bprev."""
                UO = st["UO_prev"]
                DB = dbp.tile([128, S], F32, name=f"db_{bprev}", tag="db")
                nc.vector.stream_shuffle(DB[:, :], UO[:, :], BCAST_MASK)
                OTn = otp.tile([128, S], BF16, name=f"ot_{bprev}", tag="ot")
                nc.vector._custom_dve(
                    RECIP_MUL, out=OTn[:, :], in0=UO[:, :], in1=DB[:, :],
                    s0=RC0, s1=RC1,
                )
                st["OTn_prev"] = OTn

            def wo_mm(bprev):
                gprev, pprev = bprev // 4, bprev % 4
                if pprev == 0:
                    st["WOPS_new"] = wop.tile(
                        [128, S], F32, name=f"wops_{gprev}", tag="wops"
                    )
                nc.tensor.matmul(
                    st["WOPS_new"][32 * pprev : 32 * pprev + 32, :],
                    C["woe"][:, :], st["OTn_prev"][:, :],
                    start=True, stop=True, tile_position=(0, 32 * pprev),
                    skip_group_check=True,
                )
                if pprev == 3:
                    st["WOPS"], st["XT4"] = st["WOPS_new"], st["XT4_keep"]

            # --------------------------- main loop ---------------------------
            for b in range(nb):
                g, p = b // 4, b % 4
                if p == 0:
                    U = U2[g % 2]
                    if g > 0:  # group 0's loads were issued in the preamble
                        for pp in range(4):
                            nc.sync.dma_start(
                                out=U[32 * pp : 32 * pp + 32, :, 0:D],
                                in_=x_in[4 * g + pp].rearrange(
                                    "(f c) d -> c f d", c=32
                                ),
                            )
                    XT4 = xtp.tile([128, S], F32, name=f"xt4_{g}", tag="xt")
                    nc.vector.transpose(XT4[:, :], U.rearrange("P a c -> P (a c)"))
                    XT4b = xbp.tile([128, S], BF16, name=f"xtb_{g}", tag="xb")
                    nc.vector.tensor_copy(XT4b[:, :], XT4[:, :])
                    st["XT4_new"], st["XT4b"] = XT4, XT4b

                XT4b = st["XT4b"]
                # Q/K projections -> one merged bf16 copy
                ps_qk = scp.tile([128, 2 * S], F32, name=f"qk_{b}", tag="sc")
                nc.tensor.matmul(
                    ps_qk[:, 0:S], C["wqe"][32 * p : 32 * p + D, :],
                    XT4b[32 * p : 32 * p + D, :],
                    start=True, stop=True, tile_position=(32 * p, 0),
                )
                nc.tensor.matmul(
                    ps_qk[:, S : 2 * S], C["wke"][32 * p : 32 * p + D, :],
                    XT4b[32 * p : 32 * p + D, :],
                    start=True, stop=True, tile_position=(32 * p, 0),
                )
                # previous batch's AV last chunk + softmax tail fills the gap
                if b > 0:
                    Ep, UOp, VSBp = st["E_prev"], st["UO_prev"], st["VSB_prev"]
                    for h in range(4):
                        nc.tensor.matmul(
                            UOp[32 * h : 32 * h + 32, :],
                            VSBp[:, 3, 32 * h : 32 * h + 32],
                            Ep[:, 3, h, :],
                            start=False, stop=True,
                            tile_position=(0, 32 * h),
                            skip_group_check=True,
                        )
                QKb = qkbp.tile([128, 2 * S], BF16, name=f"qkb_{b}", tag="qkb")
                nc.scalar.copy(QKb[:, :], ps_qk[:, :])
                if b > 0:
                    softmax_tail(b - 1)

                # scores + exp + AV (av(t) emitted after sc(t+1))
                E = ep.tile([128, 4, 4, S], BF16, name=f"e_{b}", tag="e")

                def sc_t(t):
                    for slot in range(2):
                        h0 = 2 * slot
                        SC = scp.tile([128, 2 * S], F32,
                                      name=f"sc{b}_{t}_{slot}", tag="sc")
                        for hh in range(2):
                            h = h0 + hh
                            nc.tensor.matmul(
                                SC[:, S * hh : S * (hh + 1)],
                                QKb[32 * h : 32 * h + HD,
                                    S + 128 * t : S + 128 * (t + 1)],
                                QKb[32 * h : 32 * h + HD, 0:S],
                                start=True, stop=True,
                                tile_position=(32 * h, 0),
                            )
                        edst = E[:, t, h0 : h0 + 2, :]
                        if EXP_ASSIGN[b % 2][2 * t + slot] == "A":
                            nc.scalar.activation(
                                edst, SC[:, :], AF.Exp, scale=float(SCALE)
                            )
                        else:
                            nc.vector._custom_dve(
                                EXP64, out=edst, in0=SC[:, :], s0=C0EXP
                            )

                def av_t(t):
                    for h in range(4):
                        nc.tensor.matmul(
                            UO[32 * h : 32 * h + 32, :],
                            Vsb[:, t, 32 * h : 32 * h + 32],
                            E[:, t, h, :],
                            start=(t == 0), stop=False,
                            tile_position=(0, 32 * h),
                            skip_group_check=True,
                        )

                sc_t(0)
                # V natural layout into the UO bank (before av(0) clears it)
                UO = uop.tile([128, S], F32, name=f"uo_{b}", tag="uo")
                for cch in range(4):
                    nc.tensor.matmul(
                        UO[:, 32 * cch : 32 * cch + 32],
                        XT4b[32 * p : 32 * p + D, 128 * cch : 128 * (cch + 1)],
                        C["wve"][32 * p : 32 * p + D, :],
                        start=True, stop=True, tile_position=(32 * p, 0),
                    )
                Vsb = VSB2[b % 2]
                nc.vector.tensor_copy(
                    Vsb.rearrange("P t (h m) -> P t h m", m=32)[:, :, :, 1 : 1 + HD],
                    UO.rearrange("P (c x) -> P c x", x=32)[:, 0:4, 0:D].rearrange(
                        "P c (h m) -> P c h m", m=HD
                    ),
                )
                sc_t(1)
                if b > 0:
                    wo_mm(b - 1)
                av_t(0)
                sc_t(2)
                av_t(1)
                sc_t(3)
                av_t(2)
                # av(3) is emitted at the start of the next batch

                run_tails(g, p)
                st["E_prev"], st["UO_prev"], st["VSB_prev"] = E, UO, Vsb
                if p == 0:
                    st["XT4_keep"] = st["XT4_new"]

            # drain: last batch's av(3) + softmax tail + wo, then tail slots
            b = nb
            Ep, UOp, VSBp = st["E_prev"], st["UO_prev"], st["VSB_prev"]
            for h in range(4):
                nc.tensor.matmul(
                    UOp[32 * h : 32 * h + 32, :],
                    VSBp[:, 3, 32 * h : 32 * h + 32],
                    Ep[:, 3, h, :],
                    start=False, stop=True,
                    tile_position=(0, 32 * h),
                    skip_group_check=True,
                )
            softmax_tail(nb - 1)
            wo_mm(nb - 1)
            for vb in range(nb, nb + 8):
                gg, pp = vb // 4, vb % 4
                if gg >= 1 and gg - 1 < ngroups and vb < nb + 4:
                    SLOTS[pp](gg - 1)
                if gg >= 2 and gg - 2 < ngroups:
                    SLOTS[4 + pp](gg - 2)
    nc.compile()
    return nc


_NC_CACHE: dict = {}


def _get_nc(nb: int, trivial_affine: bool = True) -> bass.Bass:
    key = (nb, trivial_affine)
    if key not in _NC_CACHE:
        _NC_CACHE[key] = build_nc(nb, trivial_affine)
    return _NC_CACHE[key]


def kernel(x, Wq, Wk, Wv, Wo, W1, W2, g1, b1, g2, b2):
    x = np.asarray(x, np.float32)
    args = [np.asarray(a, np.float32) for a in (Wq, Wk, Wv, Wo, W1, W2, g1, b1, g2, b2)]
    consts = _host_consts(*args)
    g1a, b1a, g2a, b2a = args[6], args[7], args[8], args[9]
    trivial = bool(
        np.all(g1a == 1.0) and np.all(b1a == 0.0)
        and np.all(g2a == 1.0) and np.all(b2a == 0.0)
    )
    nc = _get_nc(NB, trivial)
    in_maps = []
    for c in range(NCORES):
        m = {"x": np.ascontiguousarray(x[c * NB : (c + 1) * NB])}
        m.update(consts)
        in_maps.append(m)
    res = run_bass_kernel_spmd(nc, in_maps, list(range(NCORES)))
    return np.concatenate([r["out"] for r in res.results], axis=0)
